# revision 17
# baseline (speedup 1.0000x reference)
"""AFNO2D Trainium2 kernel: rfft2 -> block-diag complex MLP -> irfft2.

Sharding: one channel block (96 ch) per core x 8 cores. FFTs are per-channel
and the MLP is block-diagonal, so there is no cross-core communication.

All transforms are DFT matmuls (H=W=128 matches the PE array). Corner turns
between the W-axis and H-axis contractions go through a DRAM bounce + xbar
DMA transpose, split in halves so downstream work starts earlier. Biases
enter as rank-1 matmuls accumulated in PSUM. Softshrink drains PSUM once
(f32->bf16) and finishes with packed all-SBUF DVE ops (4x mode).

T1-side slot order is kw-paired: slot 2k = re(k), 2k+1 = im(k) for k<32 in
the first half, same for k in 32..63 in the second half; re(64) rides the
unused im(0) slot. T2-side keeps the grouped order [re 0..64 | im 1..63].
"""
import os
import sys
import numpy as np
import ml_dtypes

sys.path.insert(0, "/opt/trn_rl_repo")

B, H, W, C = 4, 128, 128, 768
NB, BS = 8, C // 8          # 8 blocks x 96 channels
WF = 65                     # rfft width
LAM = 0.01                  # softshrink
N = 128
SQ = float(np.sqrt(N))
BF16 = ml_dtypes.bfloat16

# paired slot order (T1 side): half A = kw 0..31 (+ re64 on slot 1),
# half B = kw 32..63.  kw=64 has zero imag (real input), handled "single".
PSLOT_RE = {}
PSLOT_IM = {}
for k in range(32):
    PSLOT_RE[k] = 2 * k
    if k >= 1:
        PSLOT_IM[k] = 2 * k + 1
for k in range(32, 64):
    PSLOT_RE[k] = 2 * (k - 32) + 64
    PSLOT_IM[k] = 2 * (k - 32) + 65
PSLOT_RE[64] = 1
KW_A = list(range(32)) + [64]   # uses slots [0:64] (+slot 1)
KW_B = list(range(32, 64))      # uses slots [64:128]

_CACHE = {}


def _dft_mats():
    idx = np.arange(N)
    ang = 2 * np.pi * np.outer(idx, idx) / N
    # fw: [w, slot] with the paired slot order above
    fw = np.zeros((N, N), np.float32)
    for k, s in PSLOT_RE.items():
        fw[:, s] = np.cos(ang[:, k]) / SQ
    for k, s in PSLOT_IM.items():
        fw[:, s] = -np.sin(ang[:, k]) / SQ
    gr = (np.cos(ang) / SQ).astype(np.float32)   # [h, kh]
    gi = (-np.sin(ang) / SQ).astype(np.float32)
    g = np.concatenate([gr, gi], axis=1)         # [h, 256]
    gn = np.concatenate([-gi, gr], axis=1)
    # rw: [slot_grouped, w]: rows 0..64 = re(kw), rows 65..127 = im(kw 1..63)
    rw = np.zeros((N, N), np.float32)
    wgt = np.full(65, 2.0, np.float32); wgt[0] = 1.0; wgt[64] = 1.0
    rw[:65, :] = (wgt[:, None] * np.cos(ang[:65, :])) / SQ
    rw[65:, :] = (-2.0 * np.sin(ang[1:64, :])) / SQ
    return fw, g, gn, rw


def _build():
    from concourse import bass, bacc, tile, mybir
    from contextlib import ExitStack

    bf = mybir.dt.bfloat16
    f32 = mybir.dt.float32
    AF = mybir.ActivationFunctionType
    ALU = mybir.AluOpType

    nc = bacc.Bacc("TRN2", target_bir_lowering=False, debug=False,
                   num_devices=8)

    x_d = nc.dram_tensor("x", [B, W, BS, H], bf, kind="ExternalInput").ap()
    fw_d = nc.dram_tensor("fw", [128, 128], bf, kind="ExternalInput").ap()
    g_d = nc.dram_tensor("g", [128, 256], bf, kind="ExternalInput").ap()
    gn_d = nc.dram_tensor("gn", [128, 256], bf, kind="ExternalInput").ap()
    rw_d = nc.dram_tensor("rw", [128, 128], bf, kind="ExternalInput").ap()
    w1r_d = nc.dram_tensor("w1r", [96, 96], bf, kind="ExternalInput").ap()
    w1i_d = nc.dram_tensor("w1i", [96, 96], bf, kind="ExternalInput").ap()
    w1in_d = nc.dram_tensor("w1in", [96, 96], bf, kind="ExternalInput").ap()
    b1_d = nc.dram_tensor("b1c", [96, 2], f32, kind="ExternalInput").ap()
    w2e_d = nc.dram_tensor("w2e", [97, 192], bf, kind="ExternalInput").ap()
    w2n_d = nc.dram_tensor("w2n", [96, 192], bf, kind="ExternalInput").ap()
    out_d = nc.dram_tensor("out", [B, W, H, BS], bf, kind="ExternalOutput").ap()

    with tile.TileContext(nc) as tc:
        with ExitStack() as ctx:
            pconst = ctx.enter_context(tc.tile_pool(name="const", bufs=1))
            pin = ctx.enter_context(tc.tile_pool(name="pin", bufs=4))
            pspec = ctx.enter_context(tc.tile_pool(name="pspec", bufs=2))
            pspect = ctx.enter_context(tc.tile_pool(name="pspect", bufs=1))
            pbig = ctx.enter_context(tc.tile_pool(name="pbig", bufs=2))
            psml = ctx.enter_context(tc.tile_pool(name="psml", bufs=1))
            praw = ctx.enter_context(tc.tile_pool(name="praw", bufs=2))
            pout = ctx.enter_context(tc.tile_pool(name="pout", bufs=2))
            ptmp = ctx.enter_context(tc.tile_pool(name="ptmp", bufs=2))
            pps = ctx.enter_context(tc.tile_pool(name="pps", bufs=4, space="PSUM"))
            pdram = ctx.enter_context(tc.tile_pool(name="pdram", bufs=2, space="DRAM"))

            _cn = [0]
            def const(ap_d, shape, dtype=bf):
                _cn[0] += 1
                t = pconst.tile(list(shape), dtype, tag=f"const{_cn[0]}")
                nc.sync.dma_start(out=t[:], in_=ap_d)
                return t

            fw_s = const(fw_d, (128, 128))
            g_s = const(g_d, (128, 256))
            gn_s = const(gn_d, (128, 256))
            rw_s = const(rw_d, (128, 128))
            w1r_s = const(w1r_d, (96, 96))
            w1i_s = const(w1i_d, (96, 96))
            w1in_s = const(w1in_d, (96, 96))
            b1_s = const(b1_d, (96, 2), f32)
            w2e_s = const(w2e_d, (97, 192))
            w2n_s = const(w2n_d, (96, 192))

            def stage_in(b):
                # load + cast f32->bf16 in DMA: four c-quarter tiles [w | c/4, 128]
                quarts = []
                for jc in range(4):
                    xh = pin.tile([128, BS // 4, 128], bf, tag="xin")
                    nc.gpsimd.dma_start(out=xh[:], in_=x_d[b, :, 24 * jc:24 * (jc + 1), :])
                    quarts.append(xh)
                return quarts

            def stage_s1(b, xb):
                # S1: rfft along W.  Y [slot | c, h]  (c-outer rhs order)
                y = pspec.tile([128, BS, 128], bf, tag="spec")
                for t in range(16):          # 16 tiles x (2 mm of 384), contiguous
                    xb_f = xb[t // 4][:].rearrange("w c h -> w (c h)")
                    ps = pps.tile([128, 1024], f32, tag="ps")
                    for j in range(2):
                        q = (2 * t + j) * 384 - (t // 4) * 3072
                        sl = slice(q, q + 384)
                        nc.tensor.matmul(ps[:, j * 512:j * 512 + 384], fw_s[:],
                                         xb_f[:, sl], start=True, stop=True)
                    src_ = ps[:].rearrange("k (g x) -> k g x", g=2)[:, :, :384]
                    dst = y[:, 6 * t:6 * t + 6, :].rearrange("k (g c) h -> k g (c h)", g=2)
                    eng = nc.vector.tensor_copy if t % 2 == 0 else nc.scalar.copy
                    eng(dst, src_)
                return y

            def stage_t1(b, y):
                # T1: DRAM bounce + xbar transpose, split in slot-halves
                # with separate tiles so S2 deps are half-precise.
                yt = []
                for jq in range(2):
                    ytq = pspect.tile([128, 64, BS], bf, tag=f"spect{jq}")
                    scr = pdram.tile([64, BS, 128], bf, tag=f"scr1{jq}")
                    eng = nc.gpsimd if jq % 2 == 0 else nc.sync
                    eng.dma_start(out=scr[:], in_=y[64 * jq:64 * (jq + 1)])
                    nc.sync.dma_start_transpose(
                        out=ytq[:].rearrange("h k c -> h (k c)"),
                        in_=scr[:].rearrange("k c h -> (k c) h"))
                    yt.append(ytq)
                return yt

            def stage_mid_half(b, yt, s, kws):
                kwn = len(kws)
                # S2: full DFT along H (complex).  x2h [c | ri, kwi, kh]
                def yts(slot):
                    return yt[slot // 64][:, slot % 64, :]

                x2h = pbig.tile([96, 2, kwn, 128], bf, tag="big")
                for g0 in range(0, kwn, 4):
                    sub = kws[g0:g0 + 4]
                    ps = pps.tile([128, 1024], f32, tag="ps")
                    for j, kw in enumerate(sub):
                        o = ps[:96, j * 256:(j + 1) * 256]
                        single = kw in (0, 64)
                        nc.tensor.matmul(o, yts(PSLOT_RE[kw]), g_s[:],
                                         start=True, stop=single)
                        if not single:
                            nc.tensor.matmul(o, yts(PSLOT_IM[kw]), gn_s[:],
                                             start=False, stop=True)
                    nk = len(sub)
                    src_ = ps[:96, :nk * 256].rearrange(
                        "c (k r x) -> c k r x", k=nk, r=2)
                    dst = x2h[:, :, g0:g0 + nk, :].rearrange("c r k x -> c k r x")
                    eng = nc.vector.tensor_copy if g0 % 8 == 0 else nc.scalar.copy
                    eng(dst, src_)

                x2r = x2h[:, 0].rearrange("c k h -> c (k h)")
                x2i = x2h[:, 1].rearrange("c k h -> c (k h)")

                # MLP1 + gelu; bias via activation bias.  zh [o | ri, kwi, kh]
                zh = pbig.tile([97, 2, kwn, 128], bf, tag="big")
                nc.vector.memset(zh[96:97, 0], 1.0)
                zr = zh[0:96, 0].rearrange("c k h -> c (k h)")
                zi = zh[0:96, 1].rearrange("c k h -> c (k h)")
                nf = kwn * 128
                for q0 in range(0, nf, 1024):
                    nsz = min(1024, nf - q0)
                    ps_r = pps.tile([128, 1024], f32, tag="ps")
                    ps_i = pps.tile([128, 1024], f32, tag="ps")
                    for sub in range(2 if nsz > 512 else 1):
                        ssz = min(512, nsz - sub * 512)
                        sl = slice(q0 + sub * 512, q0 + sub * 512 + ssz)
                        pr = ps_r[:96, sub * 512:sub * 512 + ssz]
                        pi = ps_i[:96, sub * 512:sub * 512 + ssz]
                        nc.tensor.matmul(pr, w1r_s[:], x2r[:, sl], start=True, stop=False)
                        nc.tensor.matmul(pr, w1in_s[:], x2i[:, sl], start=False, stop=True)
                        nc.tensor.matmul(pi, w1i_s[:], x2r[:, sl], start=True, stop=False)
                        nc.tensor.matmul(pi, w1r_s[:], x2i[:, sl], start=False, stop=True)
                    osl = slice(q0, q0 + nsz)
                    nc.scalar.activation(zr[:, osl], ps_r[:96, :nsz], AF.Gelu,
                                         bias=b1_s[:, 0:1])
                    nc.scalar.activation(zi[:, osl], ps_i[:96, :nsz], AF.Gelu,
                                         bias=b1_s[:, 1:2])

                # MLP2 (flipped: data as lhsT) -> raw drain per 4-kw group,
                # then softshrink via all-SBUF bf16 DVE ops (4x mode) into
                # s [kh | ri, kw, c] at the true kw position.
                for g0 in range(0, kwn, 4):
                    sub = kws[g0:g0 + 4]
                    nk = len(sub)
                    ps = pps.tile([128, 1024], f32, tag="ps")
                    for j, kw in enumerate(sub):
                        o = ps[:, j * 256:j * 256 + 192]
                        nc.tensor.matmul(o, zh[0:97, 0, g0 + j, :], w2e_s[:],
                                         start=True, stop=False)
                        nc.tensor.matmul(o, zh[0:96, 1, g0 + j, :], w2n_s[:],
                                         start=False, stop=True)
                    psv = ps[:].rearrange("p (k x) -> p k x", k=4)[:, :nk, :192] \
                        .rearrange("p k (r c) -> p k r c", r=2)
                    sraw = praw.tile([128, 4, 2, 96], bf, tag="sraw")
                    eng = nc.vector.tensor_copy if g0 % 8 == 0 else nc.scalar.copy
                    eng(sraw[:, :nk], psv)
                    tA = ptmp.tile([128, 4, 2, 96], bf, tag="tA")
                    nc.vector.tensor_scalar(tA[:, :nk], sraw[:, :nk], -LAM, LAM,
                                            op0=ALU.max, op1=ALU.min)
                    dst = s[:, :, sub[0]:sub[0] + nk, :].rearrange("p r k c -> p k r c")
                    nc.vector.tensor_sub(dst, sraw[:, :nk], tA[:, :nk])

            def stage_s5(b, s):
                s_r = s[:, 0].rearrange("p k c -> p (k c)")    # [kh | 6240]
                s_i = s[:, 1].rearrange("p k c -> p (k c)")
                # S5: inverse DFT along H.  hsb [h | c, slot_grouped]
                hsb = pspec.tile([128, BS, 128], bf, tag="spec")
                for t in range(7):           # 6 x (2 x 480) + 1 x 480
                    nch = 2 if t < 6 else 1
                    ps_r = pps.tile([128, 1024], f32, tag="ps")
                    ps_i = pps.tile([128, 1024], f32, tag="ps")
                    for j in range(nch):
                        cidx = 2 * t + j
                        sl = slice(cidx * 480, (cidx + 1) * 480)
                        pr = ps_r[:, j * 512:j * 512 + 480]
                        pi = ps_i[:, j * 512:j * 512 + 480]
                        nc.tensor.matmul(pr, g_s[:, 0:128], s_r[:, sl], start=True, stop=False)
                        nc.tensor.matmul(pr, g_s[:, 128:256], s_i[:, sl], start=False, stop=True)
                        nc.tensor.matmul(pi, gn_s[:, 0:128], s_r[:, sl], start=True, stop=False)
                        nc.tensor.matmul(pi, g_s[:, 0:128], s_i[:, sl], start=False, stop=True)
                    kw0 = t * 10
                    nkw = 10 if t < 6 else 5
                    # real -> slots kw; split c-halves across engines
                    srcr = ps_r[:].rearrange("p (g x) -> p g x", g=2)[:, :nch, :480] \
                        .rearrange("p g (k c) -> p g k c", c=96)
                    dstr = hsb[:, :, kw0:kw0 + nkw].rearrange("p c (g k) -> p g k c", g=nch)
                    nc.vector.tensor_copy(dstr[:, :, :, 0:48], srcr[:, :, :, 0:48])
                    nc.scalar.copy(dstr[:, :, :, 48:96], srcr[:, :, :, 48:96])
                    # imag -> slots 64+kw, dropping kw=0 and kw=64
                    if t == 0:
                        src_a = ps_i[:, 96:480].rearrange("p (k c) -> p k c", c=96)
                        dst_a = hsb[:, :, 65:69].rearrange("p c k -> p k c")
                        nc.scalar.copy(dst_a, src_a)
                        src_b = ps_i[:, 512:992].rearrange("p (k c) -> p k c", c=96)
                        dst_b = hsb[:, :, 69:74].rearrange("p c k -> p k c")
                        nc.vector.tensor_copy(dst_b, src_b)
                    elif t < 6:
                        srci = ps_i[:].rearrange("p (g x) -> p g x", g=2)[:, :, :480] \
                            .rearrange("p g (k c) -> p g k c", c=96)
                        dsti = hsb[:, :, 64 + kw0:64 + kw0 + 10].rearrange(
                            "p c (g k) -> p g k c", g=2)
                        nc.scalar.copy(dsti[:, :, :, 0:48], srci[:, :, :, 0:48])
                        nc.vector.tensor_copy(dsti[:, :, :, 48:96], srci[:, :, :, 48:96])
                    else:
                        src_c = ps_i[:, 0:384].rearrange("p (k c) -> p k c", c=96)
                        dst_c = hsb[:, :, 124:128].rearrange("p c k -> p k c")
                        nc.scalar.copy(dst_c, src_c)
                return hsb

            def stage_t2(b, hsb):
                # T2: DRAM bounce + xbar transpose in h-halves with
                # separate tiles -> 2x hst half [slot | 64 h, c]
                hst = []
                for jq in range(2):
                    hq = pspect.tile([128, 64, BS], bf, tag=f"hspect{jq}")
                    scr = pdram.tile([64, BS, 128], bf, tag=f"scr2{jq}")
                    eng = nc.gpsimd if jq % 2 == 0 else nc.sync
                    eng.dma_start(out=scr[:], in_=hsb[64 * jq:64 * (jq + 1)])
                    nc.sync.dma_start_transpose(
                        out=hq[:].rearrange("k h c -> k (h c)"),
                        in_=scr[:].rearrange("h c k -> (h c) k"))
                    hst.append(hq)
                return hst

            def stage_s6(b, hst, jq):
                # S6: irfft along W for one h-quarter -> [w | 32, c],
                # DMA'd out as soon as its three drains land.
                hq_f = hst[jq // 2][:].rearrange("k h c -> k (h c)")
                q0 = (jq % 2) * 3072
                ob = pout.tile([128, 32, BS], bf, tag="ob")
                ob_f = ob[:].rearrange("w h c -> w (h c)")
                for t in range(3):
                    ps = pps.tile([128, 1024], f32, tag="ps")
                    for j in range(2):
                        sl = slice(q0 + t * 1024 + j * 512, q0 + t * 1024 + (j + 1) * 512)
                        nc.tensor.matmul(ps[:, j * 512:(j + 1) * 512], rw_s[:],
                                         hq_f[:, sl], start=True, stop=True)
                    eng = nc.vector.tensor_copy if t % 2 == 0 else nc.scalar.copy
                    eng(ob_f[:, t * 1024:(t + 1) * 1024], ps[:])
                nc.gpsimd.dma_start(out=out_d[b, :, 32 * jq:32 * jq + 32, :],
                                    in_=ob[:])

            # software pipeline: fill bounce-latency of batch b with batch b+1
            # front-end work and batch b-1 back-end work.
            xb = stage_in(0)
            yt_cur = stage_t1(0, stage_s1(0, xb))
            xb = stage_in(1)
            yt_next = None
            hst_prev = None
            for b in range(B):
                s = psml.tile([128, 2, WF, 96], bf)
                stage_mid_half(b, yt_cur, s, KW_A)
                # back-end of b-1 first: frees hst before T1(b+1) needs bufs
                if b - 1 >= 0:
                    for jq in range(4):
                        stage_s6(b - 1, hst_prev, jq)
                if b + 1 < B:
                    y_n = stage_s1(b + 1, xb)
                    yt_next = stage_t1(b + 1, y_n)
                stage_mid_half(b, yt_cur, s, KW_B)
                hsb = stage_s5(b, s)
                hst_prev = stage_t2(b, hsb)
                # input prefetch last: its buffer-WAR waits must not sit in
                # front of bounce/out dispatches on the gpsimd queue
                if b + 2 < B:
                    xb = stage_in(b + 2)
                yt_cur = yt_next
            for jq in range(4):
                stage_s6(B - 1, hst_prev, jq)

    nc.compile()
    return nc


def _prep_maps(x, w1, b1, w2, b2):
    fw, g, gn, rw = _dft_mats()
    shared = {
        "fw": fw.astype(BF16), "g": g.astype(BF16), "gn": gn.astype(BF16),
        "rw": rw.astype(BF16),
    }
    maps = []
    for n in range(NB):
        m = dict(shared)
        m["x"] = np.ascontiguousarray(
            x[:, :, :, n * BS:(n + 1) * BS].transpose(0, 2, 3, 1)).astype(BF16)
        m["w1r"] = w1[0, n].astype(BF16)
        m["w1i"] = w1[1, n].astype(BF16)
        m["w1in"] = (-w1[1, n]).astype(BF16)
        m["b1c"] = np.stack([b1[0, n], b1[1, n]], axis=1).astype(np.float32)
        w2e = np.zeros((97, 192), np.float32)
        w2e[:96, :96] = w2[0, n]
        w2e[:96, 96:] = w2[1, n]
        w2e[96, :96] = b2[0, n]
        w2e[96, 96:] = b2[1, n]
        m["w2e"] = w2e.astype(BF16)
        m["w2n"] = np.concatenate([-w2[1, n], w2[0, n]], axis=1).astype(BF16)
        maps.append(m)
    return maps


def _enable_trace():
    """Install the axon NTFF profile hook that the image's antenv lacks."""
    import types
    import importlib.util
    try:
        from antenv.axon_hooks import get_axon_ntff_profile_hook  # noqa: F401
        return
    except ImportError:
        pass
    spec = importlib.util.spec_from_file_location(
        "trn_boot_mod", "/root/.axon_site/trn_agent_boot/trn_boot.py")
    tb = importlib.util.module_from_spec(spec)
    spec.loader.exec_module(tb)
    hook = tb._ntff_profile_via_ctypes("/opt/axon/libaxon_pjrt.so")
    import antenv
    ah = types.ModuleType("antenv.axon_hooks")
    ah._hook = hook
    ah.get_axon_ntff_profile_hook = lambda: ah._hook
    ah.set_axon_ntff_profile_hook = lambda h: setattr(ah, "_hook", h)
    sys.modules["antenv.axon_hooks"] = ah
    antenv.axon_hooks = ah
    import concourse.bass_utils as bu
    bu.upload_artifacts = lambda tmpdir: "local://" + str(tmpdir)


def kernel(x, w1, b1, w2, b2, _trace=False):
    from concourse.bass_utils import run_bass_kernel_spmd

    if _trace:
        _enable_trace()
    if "nc" not in _CACHE:
        _CACHE["nc"] = _build()
    nc = _CACHE["nc"]
    maps = _prep_maps(np.asarray(x), np.asarray(w1), np.asarray(b1),
                      np.asarray(w2), np.asarray(b2))
    res = run_bass_kernel_spmd(nc, maps, core_ids=list(range(8)), trace=_trace)
    _CACHE["last_result"] = res
    out = np.concatenate([res.results[i]["out"] for i in range(8)], axis=3)
    return np.ascontiguousarray(out.transpose(0, 2, 1, 3)).astype(np.float32)


# revision 25
# speedup vs baseline: 1.0148x; 1.0148x over previous
"""AFNO2D Trainium2 kernel: rfft2 -> block-diag complex MLP -> irfft2.

Sharding: one channel block (96 ch) per core x 8 cores. FFTs are per-channel
and the MLP is block-diagonal, so there is no cross-core communication.

All transforms are DFT matmuls (H=W=128 matches the PE array). Corner turns
between the W-axis and H-axis contractions go through a DRAM bounce + xbar
DMA transpose, split in halves so downstream work starts earlier. Biases
enter as rank-1 matmuls accumulated in PSUM. Softshrink drains PSUM once
(f32->bf16) and finishes with packed all-SBUF DVE ops (4x mode).

T1-side slot order is kw-paired: slot 2k = re(k), 2k+1 = im(k) for k<32 in
the first half, same for k in 32..63 in the second half; re(64) rides the
unused im(0) slot. T2-side keeps the grouped order [re 0..64 | im 1..63].
"""
import os
import sys
import numpy as np
import ml_dtypes

sys.path.insert(0, "/opt/trn_rl_repo")

B, H, W, C = 4, 128, 128, 768
NB, BS = 8, C // 8          # 8 blocks x 96 channels
WF = 65                     # rfft width
LAM = 0.01                  # softshrink
N = 128
SQ = float(np.sqrt(N))
BF16 = ml_dtypes.bfloat16

# paired slot order (T1 side): half A = kw 0..31 (+ re64 on slot 1),
# half B = kw 32..63.  kw=64 has zero imag (real input), handled "single".
PSLOT_RE = {}
PSLOT_IM = {}
for k in range(32):
    PSLOT_RE[k] = 2 * k
    if k >= 1:
        PSLOT_IM[k] = 2 * k + 1
for k in range(32, 64):
    PSLOT_RE[k] = 2 * (k - 32) + 64
    PSLOT_IM[k] = 2 * (k - 32) + 65
PSLOT_RE[64] = 1
KW_A = list(range(32)) + [64]   # uses slots [0:64] (+slot 1)
KW_B = list(range(32, 64))      # uses slots [64:128]

_CACHE = {}


def _dft_mats():
    idx = np.arange(N)
    ang = 2 * np.pi * np.outer(idx, idx) / N
    # fw: [w, slot] with the paired slot order above
    fw = np.zeros((N, N), np.float32)
    for k, s in PSLOT_RE.items():
        fw[:, s] = np.cos(ang[:, k]) / SQ
    for k, s in PSLOT_IM.items():
        fw[:, s] = -np.sin(ang[:, k]) / SQ
    gr = (np.cos(ang) / SQ).astype(np.float32)   # [h, kh]
    gi = (-np.sin(ang) / SQ).astype(np.float32)
    g = np.concatenate([gr, gi], axis=1)         # [h, 256]
    gn = np.concatenate([-gi, gr], axis=1)
    # rw: [slot_grouped, w]: rows 0..64 = re(kw), rows 65..127 = im(kw 1..63)
    rw = np.zeros((N, N), np.float32)
    wgt = np.full(65, 2.0, np.float32); wgt[0] = 1.0; wgt[64] = 1.0
    rw[:65, :] = (wgt[:, None] * np.cos(ang[:65, :])) / SQ
    rw[65:, :] = (-2.0 * np.sin(ang[1:64, :])) / SQ
    return fw, g, gn, rw


def _build():
    from concourse import bass, bacc, tile, mybir
    from contextlib import ExitStack

    bf = mybir.dt.bfloat16
    f32 = mybir.dt.float32
    AF = mybir.ActivationFunctionType
    ALU = mybir.AluOpType

    nc = bacc.Bacc("TRN2", target_bir_lowering=False, debug=False,
                   num_devices=8)

    x_d = nc.dram_tensor("x", [B, W, BS, H], bf, kind="ExternalInput").ap()
    fw_d = nc.dram_tensor("fw", [128, 128], bf, kind="ExternalInput").ap()
    g_d = nc.dram_tensor("g", [128, 256], bf, kind="ExternalInput").ap()
    gn_d = nc.dram_tensor("gn", [128, 256], bf, kind="ExternalInput").ap()
    rw_d = nc.dram_tensor("rw", [128, 128], bf, kind="ExternalInput").ap()
    w1r_d = nc.dram_tensor("w1r", [96, 96], bf, kind="ExternalInput").ap()
    w1i_d = nc.dram_tensor("w1i", [96, 96], bf, kind="ExternalInput").ap()
    w1in_d = nc.dram_tensor("w1in", [96, 96], bf, kind="ExternalInput").ap()
    b1_d = nc.dram_tensor("b1c", [96, 2], f32, kind="ExternalInput").ap()
    w2e_d = nc.dram_tensor("w2e", [97, 192], bf, kind="ExternalInput").ap()
    w2n_d = nc.dram_tensor("w2n", [96, 192], bf, kind="ExternalInput").ap()
    out_d = nc.dram_tensor("out", [B, W, H, BS], bf, kind="ExternalOutput").ap()

    with tile.TileContext(nc) as tc:
        with ExitStack() as ctx:
            pconst = ctx.enter_context(tc.tile_pool(name="const", bufs=1))
            pin = ctx.enter_context(tc.tile_pool(name="pin", bufs=4))
            pspec = ctx.enter_context(tc.tile_pool(name="pspec", bufs=2))
            pspect = ctx.enter_context(tc.tile_pool(name="pspect", bufs=2))
            pbig = ctx.enter_context(tc.tile_pool(name="pbig", bufs=2))
            psml = ctx.enter_context(tc.tile_pool(name="psml", bufs=1))
            praw = ctx.enter_context(tc.tile_pool(name="praw", bufs=2))
            pout = ctx.enter_context(tc.tile_pool(name="pout", bufs=2))
            ptmp = ctx.enter_context(tc.tile_pool(name="ptmp", bufs=2))
            pps = ctx.enter_context(tc.tile_pool(name="pps", bufs=4, space="PSUM"))
            pdram = ctx.enter_context(tc.tile_pool(name="pdram", bufs=2, space="DRAM"))

            _cn = [0]
            def const(ap_d, shape, dtype=bf):
                _cn[0] += 1
                t = pconst.tile(list(shape), dtype, tag=f"const{_cn[0]}")
                nc.sync.dma_start(out=t[:], in_=ap_d)
                return t

            fw_s = const(fw_d, (128, 128))
            g_s = const(g_d, (128, 256))
            gn_s = const(gn_d, (128, 256))
            rw_s = const(rw_d, (128, 128))
            w1r_s = const(w1r_d, (96, 96))
            w1i_s = const(w1i_d, (96, 96))
            w1in_s = const(w1in_d, (96, 96))
            b1_s = const(b1_d, (96, 2), f32)
            w2e_s = const(w2e_d, (97, 192))
            w2n_s = const(w2n_d, (96, 192))

            def stage_in(b):
                # load + cast f32->bf16 in DMA: four c-quarter tiles [w | c/4, 128]
                quarts = []
                for jc in range(4):
                    xh = pin.tile([128, BS // 4, 128], bf, tag="xin")
                    nc.gpsimd.dma_start(out=xh[:], in_=x_d[b, :, 24 * jc:24 * (jc + 1), :])
                    quarts.append(xh)
                return quarts

            def stage_s1(b, xb):
                # S1: rfft along W.  Y [slot | c, h]  (c-outer rhs order)
                y = pspec.tile([128, BS, 128], bf, tag="spec")
                for t in range(16):          # 16 tiles x (2 mm of 384), contiguous
                    xb_f = xb[t // 4][:].rearrange("w c h -> w (c h)")
                    ps = pps.tile([128, 1024], f32, tag="ps")
                    for j in range(2):
                        q = (2 * t + j) * 384 - (t // 4) * 3072
                        sl = slice(q, q + 384)
                        nc.tensor.matmul(ps[:, j * 512:j * 512 + 384], fw_s[:],
                                         xb_f[:, sl], start=True, stop=True)
                    src_ = ps[:].rearrange("k (g x) -> k g x", g=2)[:, :, :384]
                    dst = y[:, 6 * t:6 * t + 6, :].rearrange("k (g c) h -> k g (c h)", g=2)
                    eng = nc.vector.tensor_copy if t % 2 == 0 else nc.scalar.copy
                    eng(dst, src_)
                return y

            def stage_t1(b, y):
                # T1: DRAM bounce + xbar transpose, split in slot-halves
                yt = pspect.tile([128, 128, BS], bf, tag="spect")
                for jh in range(2):
                    scr = pdram.tile([64, BS, 128], bf, tag=f"scr1{jh}")
                    eng = nc.gpsimd if jh % 2 == 0 else nc.sync
                    eng.dma_start(out=scr[:], in_=y[64 * jh:64 * (jh + 1)])
                    nc.sync.dma_start_transpose(
                        out=yt[:, 64 * jh:64 * (jh + 1), :].rearrange("h k c -> h (k c)"),
                        in_=scr[:].rearrange("k c h -> (k c) h"))
                return yt

            def stage_mid_half(b, yt, s, kws):
                kwn = len(kws)
                # S2: full DFT along H (complex).  x2h [c | ri, kwi, kh]
                def yts(slot):
                    return yt[:, slot, :]

                x2h = pbig.tile([96, 2, kwn, 128], bf, tag="big")
                for g0 in range(0, kwn, 4):
                    sub = kws[g0:g0 + 4]
                    ps = pps.tile([128, 1024], f32, tag="ps")
                    for j, kw in enumerate(sub):
                        o = ps[:96, j * 256:(j + 1) * 256]
                        single = kw in (0, 64)
                        nc.tensor.matmul(o, yts(PSLOT_RE[kw]), g_s[:],
                                         start=True, stop=single)
                        if not single:
                            nc.tensor.matmul(o, yts(PSLOT_IM[kw]), gn_s[:],
                                             start=False, stop=True)
                    nk = len(sub)
                    src_ = ps[:96, :nk * 256].rearrange(
                        "c (k r x) -> c k r x", k=nk, r=2)
                    dst = x2h[:, :, g0:g0 + nk, :].rearrange("c r k x -> c k r x")
                    eng = nc.vector.tensor_copy if g0 % 8 == 4 else nc.scalar.copy
                    eng(dst, src_)

                x2r = x2h[:, 0].rearrange("c k h -> c (k h)")
                x2i = x2h[:, 1].rearrange("c k h -> c (k h)")

                # MLP1 + gelu; bias via activation bias.  zh [o | ri, kwi, kh]
                zh = pbig.tile([97, 2, kwn, 128], bf, tag="big")
                nc.vector.memset(zh[96:97, 0], 1.0)
                zr = zh[0:96, 0].rearrange("c k h -> c (k h)")
                zi = zh[0:96, 1].rearrange("c k h -> c (k h)")
                nf = kwn * 128
                for q0 in range(0, nf, 1024):
                    nsz = min(1024, nf - q0)
                    ps_r = pps.tile([128, 1024], f32, tag="ps")
                    ps_i = pps.tile([128, 1024], f32, tag="ps")
                    for sub in range(2 if nsz > 512 else 1):
                        ssz = min(512, nsz - sub * 512)
                        sl = slice(q0 + sub * 512, q0 + sub * 512 + ssz)
                        pr = ps_r[:96, sub * 512:sub * 512 + ssz]
                        pi = ps_i[:96, sub * 512:sub * 512 + ssz]
                        nc.tensor.matmul(pr, w1r_s[:], x2r[:, sl], start=True, stop=False)
                        nc.tensor.matmul(pr, w1in_s[:], x2i[:, sl], start=False, stop=True)
                        nc.tensor.matmul(pi, w1i_s[:], x2r[:, sl], start=True, stop=False)
                        nc.tensor.matmul(pi, w1r_s[:], x2i[:, sl], start=False, stop=True)
                    osl = slice(q0, q0 + nsz)
                    nc.scalar.activation(zr[:, osl], ps_r[:96, :nsz], AF.Gelu,
                                         bias=b1_s[:, 0:1])
                    nc.scalar.activation(zi[:, osl], ps_i[:96, :nsz], AF.Gelu,
                                         bias=b1_s[:, 1:2])

                # MLP2 (flipped: data as lhsT) -> raw drain per 4-kw group,
                # then softshrink via all-SBUF bf16 DVE ops (4x mode) into
                # s [kh | ri, kw, c] at the true kw position.
                for g0 in range(0, kwn, 4):
                    sub = kws[g0:g0 + 4]
                    nk = len(sub)
                    ps = pps.tile([128, 1024], f32, tag="ps")
                    for j, kw in enumerate(sub):
                        o = ps[:, j * 256:j * 256 + 192]
                        nc.tensor.matmul(o, zh[0:97, 0, g0 + j, :], w2e_s[:],
                                         start=True, stop=False)
                        nc.tensor.matmul(o, zh[0:96, 1, g0 + j, :], w2n_s[:],
                                         start=False, stop=True)
                    psv = ps[:].rearrange("p (k x) -> p k x", k=4)[:, :nk, :192] \
                        .rearrange("p k (r c) -> p k r c", r=2)
                    sraw = praw.tile([128, 4, 2, 96], bf, tag="sraw")
                    eng = nc.vector.tensor_copy if g0 % 8 == 0 else nc.scalar.copy
                    eng(sraw[:, :nk], psv)
                    tA = ptmp.tile([128, 4, 2, 96], bf, tag="tA")
                    nc.vector.tensor_scalar(tA[:, :nk], sraw[:, :nk], -LAM, LAM,
                                            op0=ALU.max, op1=ALU.min)
                    dst = s[:, :, sub[0]:sub[0] + nk, :].rearrange("p r k c -> p k r c")
                    nc.vector.tensor_sub(dst, sraw[:, :nk], tA[:, :nk])

            def stage_s5(b, s):
                s_r = s[:, 0].rearrange("p k c -> p (k c)")    # [kh | 6240]
                s_i = s[:, 1].rearrange("p k c -> p (k c)")
                # S5: inverse DFT along H.  hsb [h | c, slot_grouped].
                # One-bank psum tiles per 5-kw chunk keep the psum pool from
                # convoying with other stages; drains alternate engines.
                hsb = pspec.tile([128, BS, 128], bf, tag="spec")
                for t in range(13):          # 13 x 480 = 6240
                    sl = slice(t * 480, (t + 1) * 480)
                    kw0 = t * 5
                    ps = pps.tile([128, 1024], f32, tag="ps")
                    pr = ps[:, 0:480]
                    pi = ps[:, 512:992]
                    nc.tensor.matmul(pr, g_s[:, 0:128], s_r[:, sl], start=True, stop=False)
                    nc.tensor.matmul(pr, g_s[:, 128:256], s_i[:, sl], start=False, stop=True)
                    nc.tensor.matmul(pi, gn_s[:, 0:128], s_r[:, sl], start=True, stop=False)
                    nc.tensor.matmul(pi, g_s[:, 0:128], s_i[:, sl], start=False, stop=True)
                    # real parts -> slots kw0..kw0+5
                    srcr = pr.rearrange("p (k c) -> p k c", c=96)
                    dstr = hsb[:, :, kw0:kw0 + 5].rearrange("p c k -> p k c")
                    eng, eng2 = ((nc.vector.tensor_copy, nc.scalar.copy) if t % 2 == 0
                                 else (nc.scalar.copy, nc.vector.tensor_copy))
                    eng(dstr, srcr)
                    # imag parts -> slots 64+kw, dropping kw=0 and kw=64
                    i0 = 96 if t == 0 else 0
                    ni = 384 if t in (0, 12) else 480
                    srci = pi[:, i0:i0 + ni].rearrange("p (k c) -> p k c", c=96)
                    ik0 = 64 + kw0 + (1 if t == 0 else 0)
                    dsti = hsb[:, :, ik0:ik0 + ni // 96].rearrange("p c k -> p k c")
                    eng2(dsti, srci)
                return hsb

            def stage_t2(b, hsb):
                # T2: DRAM bounce + xbar transpose in h-halves -> hst [slot | h, c]
                hst = pspect.tile([128, 128, BS], bf, tag="spect")
                for jh in range(2):
                    scr = pdram.tile([64, BS, 128], bf, tag=f"scr2{jh}")
                    eng = nc.gpsimd if jh % 2 == 0 else nc.sync
                    eng.dma_start(out=scr[:], in_=hsb[64 * jh:64 * (jh + 1)])
                    nc.sync.dma_start_transpose(
                        out=hst[:, 64 * jh:64 * (jh + 1), :].rearrange("k h c -> k (h c)"),
                        in_=scr[:].rearrange("h c k -> (h c) k"))
                return hst

            def stage_s6(b, hst, jq):
                # S6: irfft along W for one h-quarter -> [w | 32, c],
                # DMA'd out as soon as its three drains land.
                hq_f = hst[:].rearrange("k h c -> k (h c)")
                q0 = jq * 3072
                ob = pout.tile([128, 32, BS], bf, tag="ob")
                ob_f = ob[:].rearrange("w h c -> w (h c)")
                for t in range(3):
                    ps = pps.tile([128, 1024], f32, tag="ps")
                    for j in range(2):
                        sl = slice(q0 + t * 1024 + j * 512, q0 + t * 1024 + (j + 1) * 512)
                        nc.tensor.matmul(ps[:, j * 512:(j + 1) * 512], rw_s[:],
                                         hq_f[:, sl], start=True, stop=True)
                    eng = nc.vector.tensor_copy if (jq + t) % 2 == 0 else nc.scalar.copy
                    eng(ob_f[:, t * 1024:(t + 1) * 1024], ps[:])
                nc.gpsimd.dma_start(out=out_d[b, :, 32 * jq:32 * jq + 32, :],
                                    in_=ob[:])

            # software pipeline: fill bounce-latency of batch b with batch b+1
            # front-end work and batch b-1 back-end work.
            xb = stage_in(0)
            yt_cur = stage_t1(0, stage_s1(0, xb))
            xb = stage_in(1)
            yt_next = None
            hst_prev = None
            for b in range(B):
                s = psml.tile([128, 2, WF, 96], bf)
                stage_mid_half(b, yt_cur, s, KW_A)
                # back-end of b-1 first: frees hst before T1(b+1) needs bufs
                if b - 1 >= 0:
                    for jq in range(4):
                        stage_s6(b - 1, hst_prev, jq)
                if b + 1 < B:
                    y_n = stage_s1(b + 1, xb)
                    yt_next = stage_t1(b + 1, y_n)
                stage_mid_half(b, yt_cur, s, KW_B)
                hsb = stage_s5(b, s)
                hst_prev = stage_t2(b, hsb)
                # input prefetch last: its buffer-WAR waits must not sit in
                # front of bounce/out dispatches on the gpsimd queue
                if b + 2 < B:
                    xb = stage_in(b + 2)
                yt_cur = yt_next
            for jq in range(4):
                stage_s6(B - 1, hst_prev, jq)

    nc.compile()
    return nc


def _prep_maps(x, w1, b1, w2, b2):
    fw, g, gn, rw = _dft_mats()
    shared = {
        "fw": fw.astype(BF16), "g": g.astype(BF16), "gn": gn.astype(BF16),
        "rw": rw.astype(BF16),
    }
    maps = []
    for n in range(NB):
        m = dict(shared)
        m["x"] = np.ascontiguousarray(
            x[:, :, :, n * BS:(n + 1) * BS].transpose(0, 2, 3, 1)).astype(BF16)
        m["w1r"] = w1[0, n].astype(BF16)
        m["w1i"] = w1[1, n].astype(BF16)
        m["w1in"] = (-w1[1, n]).astype(BF16)
        m["b1c"] = np.stack([b1[0, n], b1[1, n]], axis=1).astype(np.float32)
        w2e = np.zeros((97, 192), np.float32)
        w2e[:96, :96] = w2[0, n]
        w2e[:96, 96:] = w2[1, n]
        w2e[96, :96] = b2[0, n]
        w2e[96, 96:] = b2[1, n]
        m["w2e"] = w2e.astype(BF16)
        m["w2n"] = np.concatenate([-w2[1, n], w2[0, n]], axis=1).astype(BF16)
        maps.append(m)
    return maps


def _enable_trace():
    """Install the axon NTFF profile hook that the image's antenv lacks."""
    import types
    import importlib.util
    try:
        from antenv.axon_hooks import get_axon_ntff_profile_hook  # noqa: F401
        return
    except ImportError:
        pass
    spec = importlib.util.spec_from_file_location(
        "trn_boot_mod", "/root/.axon_site/trn_agent_boot/trn_boot.py")
    tb = importlib.util.module_from_spec(spec)
    spec.loader.exec_module(tb)
    hook = tb._ntff_profile_via_ctypes("/opt/axon/libaxon_pjrt.so")
    import antenv
    ah = types.ModuleType("antenv.axon_hooks")
    ah._hook = hook
    ah.get_axon_ntff_profile_hook = lambda: ah._hook
    ah.set_axon_ntff_profile_hook = lambda h: setattr(ah, "_hook", h)
    sys.modules["antenv.axon_hooks"] = ah
    antenv.axon_hooks = ah
    import concourse.bass_utils as bu
    bu.upload_artifacts = lambda tmpdir: "local://" + str(tmpdir)


def kernel(x, w1, b1, w2, b2, _trace=False):
    from concourse.bass_utils import run_bass_kernel_spmd

    if _trace:
        _enable_trace()
    if "nc" not in _CACHE:
        _CACHE["nc"] = _build()
    nc = _CACHE["nc"]
    maps = _prep_maps(np.asarray(x), np.asarray(w1), np.asarray(b1),
                      np.asarray(w2), np.asarray(b2))
    res = run_bass_kernel_spmd(nc, maps, core_ids=list(range(8)), trace=_trace)
    _CACHE["last_result"] = res
    out = np.concatenate([res.results[i]["out"] for i in range(8)], axis=3)
    return np.ascontiguousarray(out.transpose(0, 2, 1, 3)).astype(np.float32)


# revision 27
# speedup vs baseline: 1.0478x; 1.0325x over previous
"""AFNO2D Trainium2 kernel: rfft2 -> block-diag complex MLP -> irfft2.

Sharding: one channel block (96 ch) per core x 8 cores. FFTs are per-channel
and the MLP is block-diagonal, so there is no cross-core communication.

All transforms are DFT matmuls (H=W=128 matches the PE array). Corner turns
between the W-axis and H-axis contractions go through a DRAM bounce + xbar
DMA transpose. Biases enter as rank-1 matmuls accumulated in PSUM.
"""
import os
import sys
import numpy as np
import ml_dtypes

sys.path.insert(0, "/opt/trn_rl_repo")

B, H, W, C = 4, 128, 128, 768
NB, BS = 8, C // 8          # 8 blocks x 96 channels
WF = 65                     # rfft width
LAM = 0.01                  # softshrink
N = 128
SQ = float(np.sqrt(N))
BF16 = ml_dtypes.bfloat16
KW_HALVES = ((0, 32), (32, 33))
SLOT_RE = {k: k for k in range(33)}
SLOT_RE.update({k: 64 + (k - 33) for k in range(33, 65)})
SLOT_IM = {k: 32 + k for k in range(1, 32)}
SLOT_IM.update({k: 96 + (k - 32) for k in range(32, 64)})

_CACHE = {}


def _dft_mats():
    idx = np.arange(N)
    ang = 2 * np.pi * np.outer(idx, idx) / N
    # kw_ri slots: half-A = [re 0..32 | im 1..31], half-B = [re 33..64 | im 32..63]
    fw = np.zeros((N, N), np.float32)            # [w, kw_ri]
    for k in range(65):
        fw[:, SLOT_RE[k]] = np.cos(ang[:, k]) / SQ
    for k in range(1, 64):
        fw[:, SLOT_IM[k]] = -np.sin(ang[:, k]) / SQ
    gr = (np.cos(ang) / SQ).astype(np.float32)   # [h, kh]
    gi = (-np.sin(ang) / SQ).astype(np.float32)
    g = np.concatenate([gr, gi], axis=1)         # [h, 256]
    gn = np.concatenate([-gi, gr], axis=1)
    hc = (np.cos(ang) / SQ).astype(np.float32)   # [kh, h]
    hs = (np.sin(ang) / SQ).astype(np.float32)
    rw = np.zeros((N, N), np.float32)            # [kw_ri, w]
    wgt = np.full(65, 2.0, np.float32); wgt[0] = 1.0; wgt[64] = 1.0
    rw[:65, :] = (wgt[:, None] * np.cos(ang[:65, :])) / SQ
    rw[65:, :] = (-2.0 * np.sin(ang[1:64, :])) / SQ
    return fw, g, gn, hc, hs, (-hs).copy(), rw


def _build():
    from concourse import bass, bacc, tile, mybir
    from contextlib import ExitStack

    bf = mybir.dt.bfloat16
    f32 = mybir.dt.float32
    AF = mybir.ActivationFunctionType
    ALU = mybir.AluOpType

    nc = bacc.Bacc("TRN2", target_bir_lowering=False, debug=False,
                   num_devices=8)

    x_d = nc.dram_tensor("x", [B, W, BS, H], bf, kind="ExternalInput").ap()
    fw_d = nc.dram_tensor("fw", [128, 128], bf, kind="ExternalInput").ap()
    g_d = nc.dram_tensor("g", [128, 256], bf, kind="ExternalInput").ap()
    gn_d = nc.dram_tensor("gn", [128, 256], bf, kind="ExternalInput").ap()
    rw_d = nc.dram_tensor("rw", [128, 128], bf, kind="ExternalInput").ap()
    w1r_d = nc.dram_tensor("w1r", [96, 96], bf, kind="ExternalInput").ap()
    w1i_d = nc.dram_tensor("w1i", [96, 96], bf, kind="ExternalInput").ap()
    w1in_d = nc.dram_tensor("w1in", [96, 96], bf, kind="ExternalInput").ap()
    b1_d = nc.dram_tensor("b1c", [96, 2], f32, kind="ExternalInput").ap()
    w2e_d = nc.dram_tensor("w2e", [97, 192], bf, kind="ExternalInput").ap()
    w2n_d = nc.dram_tensor("w2n", [96, 192], bf, kind="ExternalInput").ap()
    out_d = nc.dram_tensor("out", [B, W, H, BS], bf, kind="ExternalOutput").ap()

    with tile.TileContext(nc) as tc:
        with ExitStack() as ctx:
            pconst = ctx.enter_context(tc.tile_pool(name="const", bufs=1))
            pin = ctx.enter_context(tc.tile_pool(name="pin", bufs=2))
            pspec = ctx.enter_context(tc.tile_pool(name="pspec", bufs=2))
            pspect = ctx.enter_context(tc.tile_pool(name="pspect", bufs=2))
            pbig = ctx.enter_context(tc.tile_pool(name="pbig", bufs=2))
            psml = ctx.enter_context(tc.tile_pool(name="psml", bufs=1))
            pout = ctx.enter_context(tc.tile_pool(name="pout", bufs=2))
            ptmp = ctx.enter_context(tc.tile_pool(name="ptmp", bufs=2))
            pps = ctx.enter_context(tc.tile_pool(name="pps", bufs=4, space="PSUM"))
            pdram = ctx.enter_context(tc.tile_pool(name="pdram", bufs=2, space="DRAM"))

            _cn = [0]
            def const(ap_d, shape, dtype=bf):
                _cn[0] += 1
                t = pconst.tile(list(shape), dtype, tag=f"const{_cn[0]}")
                nc.sync.dma_start(out=t[:], in_=ap_d)
                return t

            fw_s = const(fw_d, (128, 128))
            g_s = const(g_d, (128, 256))
            gn_s = const(gn_d, (128, 256))
            rw_s = const(rw_d, (128, 128))
            w1r_s = const(w1r_d, (96, 96))
            w1i_s = const(w1i_d, (96, 96))
            w1in_s = const(w1in_d, (96, 96))
            b1_s = const(b1_d, (96, 2), f32)
            w2e_s = const(w2e_d, (97, 192))
            w2n_s = const(w2n_d, (96, 192))

            def stage_in(b):
                # load + cast f32->bf16 in DMA: two c-half tiles [w | c/2, 128]
                halves = []
                for jc in range(2):
                    xh = pin.tile([128, BS // 2, 128], bf, tag="xin")
                    nc.gpsimd.dma_start(out=xh[:], in_=x_d[b, :, 48 * jc:48 * (jc + 1), :])
                    halves.append(xh)
                return halves

            def stage_s1(b, xb):
                # S1: rfft along W.  Y [kw_ri | c, h]  (c-outer rhs order)
                y = pspec.tile([128, BS, 128], bf, tag="spec")
                for t in range(16):          # 16 tiles x (2 mm of 384), contiguous
                    xb_f = xb[t // 8][:].rearrange("w c h -> w (c h)")
                    ps = pps.tile([128, 1024], f32, tag="ps")
                    for j in range(2):
                        q = (2 * t + j) * 384 - (t // 8) * 6144
                        sl = slice(q, q + 384)
                        nc.tensor.matmul(ps[:, j * 512:j * 512 + 384], fw_s[:],
                                         xb_f[:, sl], start=True, stop=True)
                    src_ = ps[:].rearrange("k (g x) -> k g x", g=2)[:, :, :384]
                    dst = y[:, 6 * t:6 * t + 6, :].rearrange("k (g c) h -> k g (c h)", g=2)
                    eng = nc.vector.tensor_copy if t % 2 == 0 else nc.scalar.copy
                    eng(dst, src_)
                return y

            def stage_t1(b, y):
                # T1: DRAM bounce + xbar transpose -> yt [h | kw_ri, c]
                scr1 = pdram.tile([128, BS, 128], bf, tag="scr")
                nc.gpsimd.dma_start(out=scr1[:], in_=y[:])
                yt = pspect.tile([128, 128, BS], bf, tag="spect")
                nc.sync.dma_start_transpose(
                    out=yt[:].rearrange("h k c -> h (k c)"),
                    in_=scr1[:].rearrange("k c h -> (k c) h"))
                return yt

            def stage_mid_half(b, yt, s, kw_base, kwn):
                if True:
                    # S2: full DFT along H (complex).  x2h [c | ri, kw, kh]
                    x2h = pbig.tile([96, 2, kwn, 128], bf, tag="big")
                    for g0 in range(0, kwn, 4):
                        kws = list(range(kw_base + g0, kw_base + min(g0 + 4, kwn)))
                        ps = pps.tile([128, 1024], f32, tag="ps")
                        for j, kw in enumerate(kws):
                            o = ps[:96, j * 256:(j + 1) * 256]
                            single = kw in (0, 64)
                            nc.tensor.matmul(o, yt[:, SLOT_RE[kw], :], g_s[:],
                                             start=True, stop=single)
                            if not single:
                                nc.tensor.matmul(o, yt[:, SLOT_IM[kw], :], gn_s[:],
                                                 start=False, stop=True)
                        nk = len(kws)
                        src_ = ps[:96, :nk * 256].rearrange(
                            "c (k r x) -> c k r x", k=nk, r=2)
                        dst = x2h[:, :, g0:g0 + nk, :].rearrange("c r k x -> c k r x")
                        eng = nc.vector.tensor_copy if g0 % 16 == 0 else nc.scalar.copy
                        eng(dst, src_)

                    x2r = x2h[:, 0].rearrange("c k h -> c (k h)")
                    x2i = x2h[:, 1].rearrange("c k h -> c (k h)")

                    # MLP1 + gelu; bias via rank-1 matmul.  zh [o | ri, kw, kh]
                    zh = pbig.tile([97, 2, kwn, 128], bf, tag="big")
                    nc.gpsimd.memset(zh[96:97, 0], 1.0)
                    zr = zh[0:96, 0].rearrange("c k h -> c (k h)")
                    zi = zh[0:96, 1].rearrange("c k h -> c (k h)")
                    nf = kwn * 128
                    for q0 in range(0, nf, 1024):
                        nsz = min(1024, nf - q0)
                        ps_r = pps.tile([128, 1024], f32, tag="ps")
                        ps_i = pps.tile([128, 1024], f32, tag="ps")
                        for sub in range(2 if nsz > 512 else 1):
                            ssz = min(512, nsz - sub * 512)
                            sl = slice(q0 + sub * 512, q0 + sub * 512 + ssz)
                            pr = ps_r[:96, sub * 512:sub * 512 + ssz]
                            pi = ps_i[:96, sub * 512:sub * 512 + ssz]
                            nc.tensor.matmul(pr, w1r_s[:], x2r[:, sl], start=True, stop=False)
                            nc.tensor.matmul(pr, w1in_s[:], x2i[:, sl], start=False, stop=True)
                            nc.tensor.matmul(pi, w1i_s[:], x2r[:, sl], start=True, stop=False)
                            nc.tensor.matmul(pi, w1r_s[:], x2i[:, sl], start=False, stop=True)
                        osl = slice(q0, q0 + nsz)
                        nc.scalar.activation(zr[:, osl], ps_r[:96, :nsz], AF.Gelu,
                                             bias=b1_s[:, 0:1])
                        nc.scalar.activation(zi[:, osl], ps_i[:96, :nsz], AF.Gelu,
                                             bias=b1_s[:, 1:2])

                    # MLP2 (flipped: data as lhsT) + softshrink -> s [kh | ri, kw, c]
                    for g0 in range(0, kwn, 4):
                        kws = list(range(kw_base + g0, kw_base + min(g0 + 4, kwn)))
                        nk = len(kws)
                        ps = pps.tile([128, 1024], f32, tag="ps")
                        for j, kw in enumerate(kws):
                            o = ps[:, j * 256:j * 256 + 192]
                            nc.tensor.matmul(o, zh[0:97, 0, kw - kw_base, :], w2e_s[:],
                                             start=True, stop=False)
                            nc.tensor.matmul(o, zh[0:96, 1, kw - kw_base, :], w2n_s[:],
                                             start=False, stop=True)
                        psv = ps[:].rearrange("p (k x) -> p k x", k=4)[:, :nk, :192]
                        tA = ptmp.tile([128, 4, 192], bf, tag="tA")
                        nc.vector.tensor_scalar(tA[:, :nk], psv, -LAM, LAM,
                                                op0=ALU.max, op1=ALU.min)
                        dst = s[:, :, kws[0]:kws[0] + nk, :].rearrange("p r k c -> p k r c")
                        nc.vector.tensor_sub(dst,
                                             psv.rearrange("p k (r c) -> p k r c", r=2),
                                             tA[:, :nk].rearrange("p k (r c) -> p k r c", r=2))

            def stage_s5(b, s):
                s_r = s[:, 0].rearrange("p k c -> p (k c)")    # [kh | 6240]
                s_i = s[:, 1].rearrange("p k c -> p (k c)")
                # S5: inverse DFT along H.  hsb [h | c, kw_ri]
                s_r = s[:, 0].rearrange("p k c -> p (k c)")    # [kh | 6240]
                s_i = s[:, 1].rearrange("p k c -> p (k c)")
                hsb = pspec.tile([128, BS, 128], bf, tag="spec")
                for t in range(7):           # 6 x (2 x 480) + 1 x 480
                    nch = 2 if t < 6 else 1
                    ps_r = pps.tile([128, 1024], f32, tag="ps")
                    ps_i = pps.tile([128, 1024], f32, tag="ps")
                    for j in range(nch):
                        cidx = 2 * t + j
                        sl = slice(cidx * 480, (cidx + 1) * 480)
                        pr = ps_r[:, j * 512:j * 512 + 480]
                        pi = ps_i[:, j * 512:j * 512 + 480]
                        nc.tensor.matmul(pr, g_s[:, 0:128], s_r[:, sl], start=True, stop=False)
                        nc.tensor.matmul(pr, g_s[:, 128:256], s_i[:, sl], start=False, stop=True)
                        nc.tensor.matmul(pi, gn_s[:, 0:128], s_r[:, sl], start=True, stop=False)
                        nc.tensor.matmul(pi, g_s[:, 0:128], s_i[:, sl], start=False, stop=True)
                    kw0 = t * 10
                    nkw = 10 if t < 6 else 5
                    # real -> slots kw; split c-halves across engines
                    srcr = ps_r[:].rearrange("p (g x) -> p g x", g=2)[:, :nch, :480] \
                        .rearrange("p g (k c) -> p g k c", c=96)
                    dstr = hsb[:, :, kw0:kw0 + nkw].rearrange("p c (g k) -> p g k c", g=nch)
                    nc.vector.tensor_copy(dstr[:, :, :, 0:48], srcr[:, :, :, 0:48])
                    nc.scalar.copy(dstr[:, :, :, 48:96], srcr[:, :, :, 48:96])
                    # imag -> slots 64+kw, dropping kw=0 and kw=64
                    if t == 0:
                        src_a = ps_i[:, 96:480].rearrange("p (k c) -> p k c", c=96)
                        dst_a = hsb[:, :, 65:69].rearrange("p c k -> p k c")
                        nc.scalar.copy(dst_a, src_a)
                        src_b = ps_i[:, 512:992].rearrange("p (k c) -> p k c", c=96)
                        dst_b = hsb[:, :, 69:74].rearrange("p c k -> p k c")
                        nc.vector.tensor_copy(dst_b, src_b)
                    elif t < 6:
                        srci = ps_i[:].rearrange("p (g x) -> p g x", g=2)[:, :, :480] \
                            .rearrange("p g (k c) -> p g k c", c=96)
                        dsti = hsb[:, :, 64 + kw0:64 + kw0 + 10].rearrange(
                            "p c (g k) -> p g k c", g=2)
                        nc.scalar.copy(dsti[:, :, :, 0:48], srci[:, :, :, 0:48])
                        nc.vector.tensor_copy(dsti[:, :, :, 48:96], srci[:, :, :, 48:96])
                    else:
                        src_c = ps_i[:, 0:384].rearrange("p (k c) -> p k c", c=96)
                        dst_c = hsb[:, :, 124:128].rearrange("p c k -> p k c")
                        nc.scalar.copy(dst_c, src_c)
                return hsb

            def stage_t2(b, hsb):
                # T2: DRAM bounce + xbar transpose (halves) -> hst [kw_ri | h, c]
                scr2 = pdram.tile([128, BS, 128], bf, tag="scr")
                nc.gpsimd.dma_start(out=scr2[:], in_=hsb[:])
                hst = pspect.tile([128, 128, BS], bf, tag="spect")
                nc.sync.dma_start_transpose(
                    out=hst[:].rearrange("k h c -> k (h c)"),
                    in_=scr2[:].rearrange("h c k -> (h c) k"))
                return hst

            def stage_s6(b, hst):
                hst_f = hst[:].rearrange("k h c -> k (h c)")
                # S6: irfft along W -> two h-half tiles [w | 64, c]
                obs = []
                for jh in range(2):
                    ob = pout.tile([128, 64, BS], bf, tag="ob")
                    ob_f = ob[:].rearrange("w h c -> w (h c)")
                    for t in range(6):
                        tt = 6 * jh + t
                        ps = pps.tile([128, 1024], f32, tag="ps")
                        for j in range(2):
                            sl = slice(tt * 1024 + j * 512, tt * 1024 + (j + 1) * 512)
                            nc.tensor.matmul(ps[:, j * 512:(j + 1) * 512], rw_s[:],
                                             hst_f[:, sl], start=True, stop=True)
                        eng = nc.vector.tensor_copy if t % 2 == 0 else nc.scalar.copy
                        eng(ob_f[:, t * 1024:(t + 1) * 1024], ps[:])
                    obs.append(ob)
                return obs

            def stage_out(b, obs):
                for jh in range(2):
                    nc.gpsimd.dma_start(out=out_d[b, :, 64 * jh:64 * (jh + 1), :],
                                        in_=obs[jh][:])

            # software pipeline: fill bounce-latency of batch b with batch b+1
            # front-end work and batch b-1 back-end work.
            # pipeline: T1(b+1) issues mid-iteration; S6(b-1) covers T1(b+1) read;
            # midA(b+1) covers T2(b) read.
            xb = stage_in(0)
            yt_cur = stage_t1(0, stage_s1(0, xb))
            xb = stage_in(1)
            yt_next = None
            hst_prev = None
            obs = {}
            for b in range(B):
                if b + 2 < B + 1 and b + 1 < B:
                    pass
                s = psml.tile([128, 2, WF, 96], bf)
                stage_mid_half(b, yt_cur, s, *KW_HALVES[0])
                if b + 1 < B:
                    y_n = stage_s1(b + 1, xb)
                    yt_next = stage_t1(b + 1, y_n)
                if b + 2 < B:
                    xb = stage_in(b + 2)
                if b - 2 >= 0:
                    stage_out(b - 2, obs[b - 2])
                stage_mid_half(b, yt_cur, s, *KW_HALVES[1])
                if b - 1 >= 0:
                    obs[b - 1] = stage_s6(b - 1, hst_prev)
                hsb = stage_s5(b, s)
                hst_prev = stage_t2(b, hsb)
                yt_cur = yt_next
            obs[B - 1] = stage_s6(B - 1, hst_prev)
            stage_out(B - 2, obs[B - 2])
            stage_out(B - 1, obs[B - 1])

    nc.compile()
    return nc


def _prep_maps(x, w1, b1, w2, b2):
    fw, g, gn, hc, hs, hsn, rw = _dft_mats()
    shared = {
        "fw": fw.astype(BF16), "g": g.astype(BF16), "gn": gn.astype(BF16),
        "hc": hc.astype(BF16), "hs": hs.astype(BF16), "hsn": hsn.astype(BF16),
        "rw": rw.astype(BF16),
    }
    maps = []
    for n in range(NB):
        m = dict(shared)
        m["x"] = np.ascontiguousarray(
            x[:, :, :, n * BS:(n + 1) * BS].transpose(0, 2, 3, 1)).astype(BF16)
        m["w1r"] = w1[0, n].astype(BF16)
        m["w1i"] = w1[1, n].astype(BF16)
        m["w1in"] = (-w1[1, n]).astype(BF16)
        m["b1c"] = np.stack([b1[0, n], b1[1, n]], axis=1).astype(np.float32)
        w2e = np.zeros((97, 192), np.float32)
        w2e[:96, :96] = w2[0, n]
        w2e[:96, 96:] = w2[1, n]
        w2e[96, :96] = b2[0, n]
        w2e[96, 96:] = b2[1, n]
        m["w2e"] = w2e.astype(BF16)
        m["w2n"] = np.concatenate([-w2[1, n], w2[0, n]], axis=1).astype(BF16)
        maps.append(m)
    return maps


def _enable_trace():
    """Install the axon NTFF profile hook that the image's antenv lacks."""
    import types
    import importlib.util
    try:
        from antenv.axon_hooks import get_axon_ntff_profile_hook  # noqa: F401
        return
    except ImportError:
        pass
    spec = importlib.util.spec_from_file_location(
        "trn_boot_mod", "/root/.axon_site/trn_agent_boot/trn_boot.py")
    tb = importlib.util.module_from_spec(spec)
    spec.loader.exec_module(tb)
    hook = tb._ntff_profile_via_ctypes("/opt/axon/libaxon_pjrt.so")
    import antenv
    ah = types.ModuleType("antenv.axon_hooks")
    ah._hook = hook
    ah.get_axon_ntff_profile_hook = lambda: ah._hook
    ah.set_axon_ntff_profile_hook = lambda h: setattr(ah, "_hook", h)
    sys.modules["antenv.axon_hooks"] = ah
    antenv.axon_hooks = ah
    import concourse.bass_utils as bu
    bu.upload_artifacts = lambda tmpdir: "local://" + str(tmpdir)


def kernel(x, w1, b1, w2, b2, _trace=False):
    from concourse.bass_utils import run_bass_kernel_spmd

    if _trace:
        _enable_trace()
    if "nc" not in _CACHE:
        _CACHE["nc"] = _build()
    nc = _CACHE["nc"]
    maps = _prep_maps(np.asarray(x), np.asarray(w1), np.asarray(b1),
                      np.asarray(w2), np.asarray(b2))
    res = run_bass_kernel_spmd(nc, maps, core_ids=list(range(8)), trace=_trace)
    _CACHE["last_result"] = res
    out = np.concatenate([res.results[i]["out"] for i in range(8)], axis=3)
    return np.ascontiguousarray(out.transpose(0, 2, 1, 3)).astype(np.float32)

